# revision 1
# baseline (speedup 1.0000x reference)
"""Trainium2 Bass kernel for the ConvFeatureExtractor problem.

Reference computation (all f32):
    matches[f, i] = sum_j kmer_params[f, kmer_idcs[i, j], j]      # (F, M)
    probs = softmax(matches / temperature, axis=1)                # over M
    pooled = freq @ probs.T                                       # (B, F)
    profile = pooled / pooled.sum(axis=1, keepdims=True)

Shapes: B=1024, M=4096 (=4^6 kmers), F=8192 filters, K=6, 4 bases.

Kernel strategy (8 NeuronCores, filter-sharded: FL = F/8 = 1024 per core):
  * matches^T = onehot(M, 24) @ params_flat^T(24, FL) as a K=24 matmul,
    where onehot one-hot-encodes kmer_idcs (built on host from the int32
    index input; it is a pure re-encoding of that input).
  * E = exp(matches/T) unnormalized (softmax denominator deferred):
    PSUM -> ACT exp -> SBUF bf16, in (M-partition, FL-free) layout.
  * U = freq @ E^T via PE bf16 matmuls accumulating over M in PSUM.
  * Z[f] = sum_i E[i, f] via DVE accumulation over M-tiles + a ones-column
    matmul for the final 128->1 partition reduction.
  * pooled = U * (1/Z) broadcast; s_part[b] = rowsum_f(pooled) per core;
    4KB AllReduce of s over the 8 cores; profile = pooled * (1/s).
Each core returns its (B, FL) f32 slice; host concatenates along F.
"""

import os

import numpy as np
import ml_dtypes

import concourse.bass as bass  # noqa: F401  (AP types come through tile/bacc)
import concourse.tile as tile
from concourse import bacc, mybir
from concourse.bass_utils import run_bass_kernel_spmd

NCORES = 8
B = 1024           # batch
M = 4096           # 4^6 kmers
F = 8192           # filters
KMER = 6           # kmer length
NBASE = 4
KK = NBASE * KMER  # 24 flattened (base, position)
FL = F // NCORES   # 1024 filters per core

MT = M // 128      # 32 contraction tiles
BT = B // 128      # 8 batch tiles
FC = 512           # psum free chunk
NFC = FL // FC     # 2

BF16 = mybir.dt.bfloat16
F32 = mybir.dt.float32
AFT = mybir.ActivationFunctionType
ALU = mybir.AluOpType

_CACHE: dict = {}


def _body(tc, freqT, onehotT, paramsT, tempr, out):
    nc = tc.nc
    with (
        tc.tile_pool(name="res", bufs=1) as res,
        tc.tile_pool(name="pm", bufs=2, space="PSUM") as pm,
        tc.tile_pool(name="pu", bufs=2, space="PSUM") as pu,
        tc.tile_pool(name="pz", bufs=2, space="PSUM") as pz,
        tc.tile_pool(name="dram", bufs=1, space="DRAM") as dram,
        tc.tile_pool(name="outp", bufs=1) as outp,
    ):
        # ---------- small inputs / constants ----------
        oh_sb = res.tile([KK, M], BF16)
        nc.sync.dma_start(oh_sb[:], onehotT[:])
        par_sb = res.tile([KK, FL], BF16)
        nc.sync.dma_start(par_sb[:], paramsT[:])
        t_sb = res.tile([128, 1], F32)       # T replicated on host to (128,1)
        nc.sync.dma_start(t_sb[:], tempr[:])
        invt_bc = res.tile([128, 1], F32)    # per-partition 1/T activation scale
        nc.vector.reciprocal(invt_bc[:], t_sb[:])
        ones_bf = res.tile([128, 128], BF16)  # lhsT: partition-sum + broadcast
        nc.vector.memset(ones_bf[:], 1.0)

        # ---------- stream in freq^T (M, B) as 32 k-tiles ----------
        freq_sb = res.tile([128, MT * B], BF16)
        for k in range(MT):
            nc.sync.dma_start(freq_sb[:, k * B:(k + 1) * B],
                              freqT[k * 128:(k + 1) * 128, :])

        # ---------- matches^T -> E = exp(matches/T); Z accumulation ----------
        E_sb = res.tile([128, MT * FL], BF16)
        zacc = res.tile([128, FL], F32)
        nc.vector.memset(zacc[:], 0.0)
        for k in range(MT):
            for fc in range(NFC):
                pm_t = pm.tile([128, FC], F32, tag="pm")
                nc.tensor.matmul(pm_t[:],
                                 lhsT=oh_sb[:, k * 128:(k + 1) * 128],
                                 rhs=par_sb[:, fc * FC:(fc + 1) * FC],
                                 start=True, stop=True)
                nc.scalar.activation(
                    E_sb[:, k * FL + fc * FC: k * FL + (fc + 1) * FC],
                    pm_t[:], AFT.Exp, scale=invt_bc[:])
            nc.vector.tensor_add(zacc[:], zacc[:], E_sb[:, k * FL:(k + 1) * FL])

        stage = os.environ.get("KERNEL_STAGE", "")
        if stage == "1":
            # bisect: write exp(matches/T) tiles for batch-tile-shaped slices
            for b in range(BT):
                prof = outp.tile([128, FL], F32, tag="prof")
                nc.scalar.copy(prof[:], E_sb[:, b * FL:(b + 1) * FL])
                nc.sync.dma_start(out[b * 128:(b + 1) * 128, :], prof[:])
            return

        U_sb = res.tile([128, BT * FL], F32)
        s_col = res.tile([128, BT], F32)
        invz_bc = res.tile([128, FL], F32)

        zacc_bf = res.tile([128, FL], BF16)

        def z_finish():
            # ones(128,128).T @ zacc_bf = column sums broadcast to every
            # partition, as a standard-shape bf16 matmul per chunk
            nc.scalar.copy(zacc_bf[:], zacc[:])
            for fc in range(NFC):
                zbc_ps = pz.tile([128, FC], F32, tag="pz", name=f"zbc{fc}")
                nc.tensor.matmul(zbc_ps[:], lhsT=ones_bf[:],
                                 rhs=zacc_bf[:, fc * FC:(fc + 1) * FC],
                                 start=True, stop=True)
                nc.scalar.copy(zacc[:, fc * FC:(fc + 1) * FC], zbc_ps[:])
                nc.vector.reciprocal(invz_bc[:, fc * FC:(fc + 1) * FC],
                                     zacc[:, fc * FC:(fc + 1) * FC])

        # ---------- U = freq @ E^T per batch tile; scale by 1/Z; rowsums ----
        for b in range(BT):
            pu0 = pu.tile([128, FC], F32, tag="pu0")
            pu1 = pu.tile([128, FC], F32, tag="pu1")
            if os.environ.get("KERNEL_INTERLEAVE"):
                for k in range(MT):
                    lw = freq_sb[:, k * B + b * 128: k * B + (b + 1) * 128]
                    nc.tensor.matmul(pu0[:], lhsT=lw,
                                     rhs=E_sb[:, k * FL: k * FL + FC],
                                     start=(k == 0), stop=(k == MT - 1))
                    nc.tensor.matmul(pu1[:], lhsT=lw,
                                     rhs=E_sb[:, k * FL + FC: (k + 1) * FL],
                                     start=(k == 0), stop=(k == MT - 1))
            else:
                for fc, put in ((0, pu0), (1, pu1)):
                    for k in range(MT):
                        lw = freq_sb[:, k * B + b * 128: k * B + (b + 1) * 128]
                        nc.tensor.matmul(put[:], lhsT=lw,
                                         rhs=E_sb[:, k * FL + fc * FC:
                                                  k * FL + (fc + 1) * FC],
                                         start=(k == 0), stop=(k == MT - 1))
            nc.scalar.copy(U_sb[:, b * FL: b * FL + FC], pu0[:])
            nc.scalar.copy(U_sb[:, b * FL + FC: (b + 1) * FL], pu1[:])
            if stage == "2":
                nc.sync.dma_start(out[b * 128:(b + 1) * 128, :],
                                  U_sb[:, b * FL:(b + 1) * FL])
                continue
            if b == 0:
                # emitted here so PE's in-order stream hits these tiny f32
                # matmuls right when zacc's DVE chain completes
                z_finish()
            nc.vector.tensor_mul(U_sb[:, b * FL:(b + 1) * FL],
                                 U_sb[:, b * FL:(b + 1) * FL], invz_bc[:])
            nc.vector.reduce_sum(s_col[:, b:b + 1],
                                 U_sb[:, b * FL:(b + 1) * FL],
                                 axis=mybir.AxisListType.X)

        if stage == "2":
            return
        if stage == "3":
            for b in range(BT):
                nc.sync.dma_start(out[b * 128:(b + 1) * 128, :],
                                  U_sb[:, b * FL:(b + 1) * FL])
            return

        # ---------- AllReduce of per-core rowsums (4KB) ----------
        s_sum = res.tile([128, BT], F32)
        if os.environ.get("KERNEL_NO_COLLECTIVE"):
            nc.vector.tensor_scalar_mul(s_sum[:], s_col[:], float(NCORES))
        else:
            s_in = dram.tile([128, BT], F32)
            s_out = dram.tile([128, BT], F32, addr_space="Shared")
            nc.sync.dma_start(s_in[:], s_col[:])
            nc.gpsimd.collective_compute(
                "AllReduce", ALU.add,
                replica_groups=[list(range(NCORES))],
                ins=[s_in.opt()], outs=[s_out.opt()])
            nc.sync.dma_start(s_sum[:], s_out[:])
        rinv = res.tile([128, BT], F32)
        nc.vector.reciprocal(rinv[:], s_sum[:])

        # ---------- profile = pooled * (1/s); write out ----------
        for b in range(BT):
            prof = outp.tile([128, FL], F32, tag="prof")
            nc.vector.tensor_scalar_mul(prof[:], U_sb[:, b * FL:(b + 1) * FL],
                                        rinv[:, b:b + 1])
            nc.sync.dma_start(out[b * 128:(b + 1) * 128, :], prof[:])


def _build_bass():
    nc = bacc.Bacc("TRN2", target_bir_lowering=False, debug=False,
                   num_devices=NCORES)
    freqT = nc.dram_tensor("freqT", [M, B], BF16, kind="ExternalInput").ap()
    onehotT = nc.dram_tensor("onehotT", [KK, M], BF16, kind="ExternalInput").ap()
    paramsT = nc.dram_tensor("paramsT", [KK, FL], BF16, kind="ExternalInput").ap()
    tempr = nc.dram_tensor("tempr", [128, 1], F32, kind="ExternalInput").ap()
    out = nc.dram_tensor("out", [B, FL], F32, kind="ExternalOutput").ap()

    with tile.TileContext(nc) as tc:
        _body(tc, freqT, onehotT, paramsT, tempr, out)
    nc.compile()
    return nc


def _get_nc():
    if "nc" not in _CACHE:
        _CACHE["nc"] = _build_bass()
    return _CACHE["nc"]


def _prepare_in_maps(freq, kmer_params, temperature, kmer_idcs):
    freq = np.asarray(freq, dtype=np.float32)            # (B, M)
    kp = np.asarray(kmer_params, dtype=np.float32)       # (F, 4, K)
    temp = np.asarray(temperature, dtype=np.float32).reshape(-1)[:1]
    idcs = np.asarray(kmer_idcs).astype(np.int64)        # (M, K)

    assert freq.shape == (B, M) and kp.shape == (F, NBASE, KMER)
    assert idcs.shape == (M, KMER)

    # one-hot re-encoding of the index input: onehot[i, c*K + j] = 1 iff
    # kmer_idcs[i, j] == c   (params_flat[f, c*K + j] = kmer_params[f, c, j])
    onehot = np.zeros((M, NBASE, KMER), dtype=np.float32)
    onehot[np.arange(M)[:, None], idcs, np.arange(KMER)[None, :]] = 1.0
    onehotT = np.ascontiguousarray(
        onehot.reshape(M, KK).T).astype(ml_dtypes.bfloat16)

    params_flat = kp.reshape(F, KK)
    freqT = np.ascontiguousarray(freq.T).astype(ml_dtypes.bfloat16)
    tempr = np.ascontiguousarray(np.broadcast_to(temp.reshape(1, 1), (128, 1)))

    in_maps = []
    for c in range(NCORES):
        paramsT_c = np.ascontiguousarray(
            params_flat[c * FL:(c + 1) * FL].T).astype(ml_dtypes.bfloat16)
        in_maps.append({
            "freqT": freqT,
            "onehotT": onehotT,
            "paramsT": paramsT_c,
            "tempr": tempr,
        })
    return in_maps


def _run(in_maps, trace=False):
    nc = _get_nc()
    return run_bass_kernel_spmd(nc, in_maps, list(range(NCORES)), trace=trace)


def kernel(freq, kmer_params, temperature, kmer_idcs):
    in_maps = _prepare_in_maps(freq, kmer_params, temperature, kmer_idcs)
    res = _run(in_maps,
               trace=os.environ.get("KERNEL_TRACE", "") not in ("", "0"))
    _CACHE["last_result"] = res
    return np.concatenate(
        [np.asarray(res.results[c]["out"], dtype=np.float32)
         for c in range(NCORES)], axis=1)



# revision 11
# speedup vs baseline: 1.4906x; 1.4906x over previous
"""Trainium2 Bass kernel for the ConvFeatureExtractor problem.

Reference computation (all f32):
    matches[f, i] = sum_j kmer_params[f, kmer_idcs[i, j], j]      # (F, M)
    probs = softmax(matches / temperature, axis=1)                # over M
    pooled = freq @ probs.T                                       # (B, F)
    profile = pooled / pooled.sum(axis=1, keepdims=True)

Shapes: B=1024, M=4096 (=4^6 kmers), F=8192 filters, K=6, 4 bases.

Kernel strategy (8 NeuronCores, filter-sharded: FL = F/8 = 1024 per core):
  * kmer_idcs enumerates the full 4^6 product space, so E = exp(matches/T)
    factorizes as a Kronecker product over the high/low 3 digits:
    E[i, f] = A[i>>6, f] * C[i&63, f], with A/C = exp of tiny (12-contraction)
    onehot matmuls.  A is broadcast 64-fold via DRAM->SBUF replicating DMAs;
    E tiles are produced as fp8 e4m3 by DVE/GpSimd multiplies (no big PE or
    ACT work for E at all).
  * Params are pre-shifted on host by (rowmax_f - margin*T)/K where
    rowmax_f = sum_j max_c kp[f,c,j] bounds every row of matches, so
    E <= e^margin stays in fp8 range; the shift cancels in the softmax.
  * U = freq @ E^T with fp8 DoubleRow matmuls: each instruction contracts
    two 128-row k-tiles (rhs/lhsT carry a [2] pair dim), 0.5 cycles/row.
  * Z[f] = sum_i E[i, f] via an extra all-ones DoubleRow accumulator row
    (ones(128).T @ E broadcasts the column sums to every partition).
  * Evacuate U with DVE: U_sb = psum * (1/Z), then row-sums s[b].
  * 4KB AllReduce of s over the 8 cores; profile = pooled * (1/s).
Each core returns its (B, FL) f32 slice; host concatenates along F.
"""

import os

import numpy as np
import ml_dtypes

import concourse.bass as bass  # noqa: F401  (AP types come through tile/bacc)
import concourse.tile as tile
from concourse import bacc, mybir
from concourse.bass_utils import run_bass_kernel_spmd

NCORES = 8
B = 1024           # batch
M = 4096           # 4^6 kmers
F = 8192           # filters
KMER = 6           # kmer length
NBASE = 4
FL = F // NCORES   # 1024 filters per core

MT = M // 128      # 32 contraction tiles
NP2 = MT // 2      # 16 DoubleRow pair steps
BT = B // 128      # 8 batch tiles
FC = 512           # psum free chunk
MARGIN = 5.0       # exp(margin) = 148 < 240 (fp8 e4m3 max normal)
# 1: freq quantized to one fp8 term; 2: fp8 + fp8 residual (more accurate,
# doubles the pooled-matmul work)
FREQ_TERMS = int(os.environ.get("KERNEL_FREQ_TERMS", "1"))

BF16 = mybir.dt.bfloat16
F8 = mybir.dt.float8e4
F32 = mybir.dt.float32
AFT = mybir.ActivationFunctionType
ALU = mybir.AluOpType
DR = mybir.MatmulPerfMode.DoubleRow

_CACHE: dict = {}


def _body(tc, freqT, freqTb, oh3aT, par3aT, oh3cT, par3cT, tempr, out):
    nc = tc.nc
    with (
        tc.tile_pool(name="res", bufs=1) as res,
        tc.tile_pool(name="arep", bufs=4) as apool,
        tc.tile_pool(name="pm", bufs=2, space="PSUM") as pm,
        tc.tile_pool(name="pacc", bufs=1, space="PSUM") as pacc,
        tc.tile_pool(name="dram", bufs=1, space="DRAM") as dram,
        tc.tile_pool(name="outp", bufs=4) as outp,
    ):
        # ---------- small inputs / constants (sync queue, smallest first) --
        t_sb = res.tile([128, 1], F32)       # T replicated on host to (128,1)
        nc.sync.dma_start(t_sb[:], tempr[:])
        oh3a_sb = res.tile([12, 64], BF16)
        nc.sync.dma_start(oh3a_sb[:], oh3aT[:])
        oh3c_sb = res.tile([12, 128], BF16)
        nc.sync.dma_start(oh3c_sb[:], oh3cT[:])
        par3a_sb = res.tile([12, FL], BF16)
        nc.sync.dma_start(par3a_sb[:], par3aT[:])
        par3c_sb = res.tile([12, FL], BF16)
        nc.sync.dma_start(par3c_sb[:], par3cT[:])
        invt_bc = res.tile([128, 1], F32)    # per-partition 1/T activation scale
        nc.vector.reciprocal(invt_bc[:], t_sb[:])
        ones8 = res.tile([128, 2, 128], F8)  # DoubleRow all-ones lhsT pair
        nc.vector.memset(ones8[:], 1.0)

        # ---------- freq (fp8, pre-tiled on host) on the ACT dma ring ------
        freq_sb = res.tile([128, MT, B], F8)
        for c in range(4):
            nc.scalar.dma_start(freq_sb[:, c * 8:(c + 1) * 8, :],
                                freqT[:, c * 8:(c + 1) * 8, :])
        if FREQ_TERMS == 2:
            freqb_sb = res.tile([128, MT, B], F8)
            for c in range(4):
                nc.scalar.dma_start(freqb_sb[:, c * 8:(c + 1) * 8, :],
                                    freqTb[:, c * 8:(c + 1) * 8, :])

        # ---------- A (64, FL) and Crep (128, FL): tiny matmuls + exp ------
        pa_t = pm.tile([128, 1024], F32, tag="pm")
        nc.tensor.matmul(pa_t[0:64, 0:FC], lhsT=oh3a_sb[:],
                         rhs=par3a_sb[:, 0:FC], start=True, stop=True)
        nc.tensor.matmul(pa_t[0:64, FC:2 * FC], lhsT=oh3a_sb[:],
                         rhs=par3a_sb[:, FC:2 * FC], start=True, stop=True)
        A_sb = res.tile([64, FL], BF16)
        nc.scalar.activation(A_sb[:], pa_t[0:64, :], AFT.Exp,
                             scale=invt_bc[0:64, :])
        pc_t = pm.tile([128, 1024], F32, tag="pm")
        nc.tensor.matmul(pc_t[:, 0:FC], lhsT=oh3c_sb[:],
                         rhs=par3c_sb[:, 0:FC], start=True, stop=True)
        nc.tensor.matmul(pc_t[:, FC:2 * FC], lhsT=oh3c_sb[:],
                         rhs=par3c_sb[:, FC:2 * FC], start=True, stop=True)
        crep_sb = res.tile([128, FL], BF16)
        nc.scalar.activation(crep_sb[:], pc_t[:], AFT.Exp, scale=invt_bc[:])
        # A goes to DRAM so replicating (stride-0 source) DMAs can fan each
        # row out to 64 partitions (SBUF sources cannot stride-0)
        a_dram = dram.tile([64, FL], BF16)
        nc.sync.dma_start(a_dram[:], A_sb[:])

        E_sb = res.tile([128, MT, FL], F8)
        U_sb = res.tile([128, BT, FL], F32)
        invz = res.tile([128, FL], F32)
        s_col = res.tile([128, BT], F32)

        # persistent PSUM accumulators: Z row + batch-tile 0
        pz = pacc.tile([128, 1024], F32)
        pb0 = pacc.tile([128, 1024], F32)

        # ---------- phase A: E tiles by broadcast + multiply; Z, b0 -------
        def e_make(kk):
            arep_t = apool.tile([128, FL], BF16, tag="arep")
            ring = nc.sync if kk % 2 == 0 else nc.scalar
            ring.dma_start(arep_t[0:64, :],
                           a_dram[2 * kk:2 * kk + 1, :].broadcast_to([64, FL]))
            ring.dma_start(arep_t[64:128, :],
                           a_dram[2 * kk + 1:2 * kk + 2, :]
                           .broadcast_to([64, FL]))
            eng = nc.vector if kk % 2 == 0 else nc.gpsimd
            eng.tensor_mul(E_sb[:, kk, :], arep_t[:], crep_sb[:])

        def dr_step(k2):
            first, last = (k2 == 0), (k2 == NP2 - 1)
            er0 = E_sb[:, 2 * k2:2 * k2 + 2, 0:FC]
            er1 = E_sb[:, 2 * k2:2 * k2 + 2, FC:2 * FC]
            nc.tensor.matmul(pz[:, 0:FC], lhsT=ones8[:], rhs=er0,
                             perf_mode=DR, start=first, stop=last)
            nc.tensor.matmul(pz[:, FC:2 * FC], lhsT=ones8[:], rhs=er1,
                             perf_mode=DR, start=first, stop=last)
            lw = freq_sb[:, 2 * k2:2 * k2 + 2, 0:128]
            stop0 = last and FREQ_TERMS == 1
            nc.tensor.matmul(pb0[:, 0:FC], lhsT=lw, rhs=er0,
                             perf_mode=DR, start=first, stop=stop0)
            nc.tensor.matmul(pb0[:, FC:2 * FC], lhsT=lw, rhs=er1,
                             perf_mode=DR, start=first, stop=stop0)
            if FREQ_TERMS == 2:
                lwb = freqb_sb[:, 2 * k2:2 * k2 + 2, 0:128]
                nc.tensor.matmul(pb0[:, 0:FC], lhsT=lwb, rhs=er0,
                                 perf_mode=DR, start=False, stop=last)
                nc.tensor.matmul(pb0[:, FC:2 * FC], lhsT=lwb, rhs=er1,
                                 perf_mode=DR, start=False, stop=last)

        # software-pipelined: DR accumulation for pair k2-1 runs while the
        # broadcast DMAs + multiplies produce pair k2
        for k2 in range(NP2 + 1):
            if k2 < NP2:
                e_make(2 * k2)
                e_make(2 * k2 + 1)
            if k2 >= 1:
                dr_step(k2 - 1)

        # ---------- 1/Z (broadcast on all partitions by the ones matmul) ---
        nc.vector.reciprocal(invz[:, 0:FC], pz[:, 0:FC])
        nc.vector.reciprocal(invz[:, FC:2 * FC], pz[:, FC:2 * FC])

        def evac(b, psum_t):
            # U_sb = psum * (1/Z), then row sums into s_col
            nc.vector.tensor_mul(U_sb[:, b, 0:FC], psum_t[:, 0:FC],
                                 invz[:, 0:FC])
            nc.vector.tensor_mul(U_sb[:, b, FC:2 * FC], psum_t[:, FC:2 * FC],
                                 invz[:, FC:2 * FC])
            nc.vector.reduce_sum(s_col[:, b:b + 1], U_sb[:, b, :],
                                 axis=mybir.AxisListType.X)

        evac(0, pb0)

        # ---------- phase B: batch tiles 1..7 ----------
        for b in range(1, BT):
            pu_t = pm.tile([128, 1024], F32, tag="pm")
            for k2 in range(NP2):
                first, last = (k2 == 0), (k2 == NP2 - 1)
                er0 = E_sb[:, 2 * k2:2 * k2 + 2, 0:FC]
                er1 = E_sb[:, 2 * k2:2 * k2 + 2, FC:2 * FC]
                lw = freq_sb[:, 2 * k2:2 * k2 + 2, b * 128:(b + 1) * 128]
                stop0 = last and FREQ_TERMS == 1
                nc.tensor.matmul(pu_t[:, 0:FC], lhsT=lw, rhs=er0,
                                 perf_mode=DR, start=first, stop=stop0)
                nc.tensor.matmul(pu_t[:, FC:2 * FC], lhsT=lw, rhs=er1,
                                 perf_mode=DR, start=first, stop=stop0)
                if FREQ_TERMS == 2:
                    lwb = freqb_sb[:, 2 * k2:2 * k2 + 2, b * 128:(b + 1) * 128]
                    nc.tensor.matmul(pu_t[:, 0:FC], lhsT=lwb, rhs=er0,
                                     perf_mode=DR, start=False, stop=last)
                    nc.tensor.matmul(pu_t[:, FC:2 * FC], lhsT=lwb, rhs=er1,
                                     perf_mode=DR, start=False, stop=last)
            evac(b, pu_t)

        # ---------- AllReduce of per-core rowsums (4KB) ----------
        s_sum = res.tile([128, BT], F32)
        if os.environ.get("KERNEL_NO_COLLECTIVE"):
            nc.vector.tensor_scalar_mul(s_sum[:], s_col[:], float(NCORES))
        else:
            s_in = dram.tile([128, BT], F32)
            s_out = dram.tile([128, BT], F32, addr_space="Shared")
            nc.sync.dma_start(s_in[:], s_col[:])
            nc.gpsimd.collective_compute(
                "AllReduce", ALU.add,
                replica_groups=[list(range(NCORES))],
                ins=[s_in.opt()], outs=[s_out.opt()])
            nc.sync.dma_start(s_sum[:], s_out[:])
        rinv = res.tile([128, BT], F32)
        nc.vector.reciprocal(rinv[:], s_sum[:])

        # ---------- profile = pooled * (1/s); write out (2 dma rings) ------
        for b in range(BT):
            prof = outp.tile([128, FL], F32, tag="prof")
            nc.vector.tensor_scalar_mul(prof[:], U_sb[:, b, :],
                                        rinv[:, b:b + 1])
            eng = nc.sync if b % 2 == 0 else nc.scalar
            eng.dma_start(out[b * 128:(b + 1) * 128, :], prof[:])


def _build_bass():
    nc = bacc.Bacc("TRN2", target_bir_lowering=False, debug=False,
                   num_devices=NCORES)
    freqT = nc.dram_tensor("freqT", [128, MT * B], F8, kind="ExternalInput").ap()
    freqT = freqT.rearrange("p (k b) -> p k b", k=MT)
    freqTb = None
    if FREQ_TERMS == 2:
        freqTb = nc.dram_tensor("freqTb", [128, MT * B], F8,
                                kind="ExternalInput").ap()
        freqTb = freqTb.rearrange("p (k b) -> p k b", k=MT)
    oh3aT = nc.dram_tensor("oh3aT", [12, 64], BF16, kind="ExternalInput").ap()
    par3aT = nc.dram_tensor("par3aT", [12, FL], BF16, kind="ExternalInput").ap()
    oh3cT = nc.dram_tensor("oh3cT", [12, 128], BF16, kind="ExternalInput").ap()
    par3cT = nc.dram_tensor("par3cT", [12, FL], BF16, kind="ExternalInput").ap()
    tempr = nc.dram_tensor("tempr", [128, 1], F32, kind="ExternalInput").ap()
    out = nc.dram_tensor("out", [B, FL], F32, kind="ExternalOutput").ap()

    with tile.TileContext(nc) as tc:
        _body(tc, freqT, freqTb, oh3aT, par3aT, oh3cT, par3cT, tempr, out)
    nc.compile()
    return nc


def _get_nc():
    if "nc" not in _CACHE:
        _CACHE["nc"] = _build_bass()
    return _CACHE["nc"]


def _prepare_in_maps(freq, kmer_params, temperature, kmer_idcs):
    freq = np.asarray(freq, dtype=np.float32)            # (B, M)
    kp = np.asarray(kmer_params, dtype=np.float32)       # (F, 4, K)
    temp = np.asarray(temperature, dtype=np.float32).reshape(-1)[:1]
    idcs = np.asarray(kmer_idcs).astype(np.int64)        # (M, K)

    assert freq.shape == (B, M) and kp.shape == (F, NBASE, KMER)
    assert idcs.shape == (M, KMER)

    # Kronecker split of the kmer enumeration: positions 0-2 must depend
    # only on i>>6 and positions 3-5 only on i&63 (true for the full
    # product-space enumeration this module uses)
    i = np.arange(M)
    hi = idcs[(i >> 6) * 64][:, :3]                      # (M, 3) via h
    lo = idcs[i & 63][:, 3:]                             # (M, 3) via l
    assert np.array_equal(idcs[:, :3], hi) and np.array_equal(idcs[:, 3:], lo)
    hi64 = idcs[::64, :3]                                # (64, 3)
    lo64 = idcs[:64, 3:]                                 # (64, 3)

    # shift params by (rowmax - margin*T)/K: bounds E by e^margin; the
    # per-filter factor cancels in the softmax normalization
    tval = float(temp[0])
    rowmax = kp.max(axis=1).sum(axis=1)                  # (F,)
    kp_shift = kp - ((rowmax - MARGIN * tval) / KMER)[:, None, None]

    bf = ml_dtypes.bfloat16
    oh3aT = np.zeros((12, 64), dtype=np.float32)
    oh3cT = np.zeros((12, 128), dtype=np.float32)
    par3aT = np.empty((12, FL * NCORES), dtype=np.float32)
    par3cT = np.empty((12, FL * NCORES), dtype=np.float32)
    for c in range(NBASE):
        for j in range(3):
            oh3aT[c * 3 + j, :] = (hi64[:, j] == c)
            oh3cT[c * 3 + j, :] = (lo64[np.arange(128) & 63, j] == c)
            par3aT[c * 3 + j, :] = kp_shift[:, c, j]
            par3cT[c * 3 + j, :] = kp_shift[:, c, 3 + j]
    oh3aT = oh3aT.astype(bf)
    oh3cT = oh3cT.astype(bf)

    # freq^T tiled to the SBUF layout [128, MT, B] and quantized to fp8
    f8 = ml_dtypes.float8_e4m3
    ftile = np.ascontiguousarray(
        freq.T.reshape(MT, 128, B).transpose(1, 0, 2)).reshape(128, MT * B)
    freqT8 = ftile.astype(f8)
    if FREQ_TERMS == 2:
        freqT8b = (ftile - freqT8.astype(np.float32)).astype(f8)
    tempr = np.ascontiguousarray(np.broadcast_to(temp.reshape(1, 1), (128, 1)))

    in_maps = []
    for c in range(NCORES):
        im = {
            "freqT": freqT8,
            "oh3aT": oh3aT,
            "oh3cT": oh3cT,
            "par3aT": np.ascontiguousarray(
                par3aT[:, c * FL:(c + 1) * FL]).astype(bf),
            "par3cT": np.ascontiguousarray(
                par3cT[:, c * FL:(c + 1) * FL]).astype(bf),
            "tempr": tempr,
        }
        if FREQ_TERMS == 2:
            im["freqTb"] = freqT8b
        in_maps.append(im)
    return in_maps


def _run(in_maps, trace=False):
    nc = _get_nc()
    return run_bass_kernel_spmd(nc, in_maps, list(range(NCORES)), trace=trace)


def kernel(freq, kmer_params, temperature, kmer_idcs):
    in_maps = _prepare_in_maps(freq, kmer_params, temperature, kmer_idcs)
    res = _run(in_maps,
               trace=os.environ.get("KERNEL_TRACE", "") not in ("", "0"))
    _CACHE["last_result"] = res
    return np.concatenate(
        [np.asarray(res.results[c]["out"], dtype=np.float32)
         for c in range(NCORES)], axis=1)
